# revision 1
# baseline (speedup 1.0000x reference)
"""CLAHE Bass/Tile kernel for TRN2 — builder module.

Pipeline per core (one 2048x2048 image):
  Phase 1 (hist):  b = floor(256x) int16; 16+16 digit one-hots (bf16);
                   PE matmuls A^T@B accumulate 16x16 joint hist per 256x256 tile.
  Phase 2 (mid):   clip at 204, redistribute, cumsum -> per-tile LUT (int).
  Phase 3 (tables):region 4-LUT packed int32 tables, broadcast via DRAM bounce.
  Phase 4 (apply): b' = floor(255x); per-bin mask + copy_predicated chain
                   gathers packed region table; unpack; bilinear blend; store.
"""
import sys

sys.path.insert(0, "/opt/trn_rl_repo")
from contextlib import ExitStack

import concourse.bacc as bacc
import concourse.tile as tile
import concourse.bass as bass
import concourse.mybir as mybir
from concourse._compat import with_exitstack

P = 128
dt = mybir.dt
Alu = mybir.AluOpType

H = W = 2048
GH = GW = 8
TH = TW = 256
NB = 256
PIXELS = TH * TW          # 65536
MAXVAL = 204.0            # int(0.8 * 65536 // 256)
NHB = H // P              # 16 half-bands of 128 rows

# x-region of a 128-col half-span hs (region boundaries at 128+256k)
RX_OF_HS = [0, 1, 1, 2, 2, 3, 3, 4, 4, 5, 5, 6, 6, 7, 7, 8]
# y-region of a 128-row half-band h
RY_OF_HB = [(h + 1) >> 1 for h in range(NHB)]


_FLOOR_N = [0]


def floor_pass(nc, pool, y, tag, out_dtype=dt.int16):
    """Exact floor of an fp32 tile -> int tile (valid for y > -1).
    Handles unknown convert rounding: t = cvt(y); fix = (cvt_back(t) > y); b = t - fix."""
    p, f = y.shape[0], y.shape[1]
    _FLOOR_N[0] += 1
    u = _FLOOR_N[0]
    t = pool.tile([p, f], out_dtype, tag=f"{tag}_t", name=f"{tag}_t{u}")
    nc.vector.tensor_scalar(out=t[:], in0=y[:], scalar1=0.0, scalar2=None, op0=Alu.add)
    tf = pool.tile([p, f], dt.float32, tag=f"{tag}_tf", name=f"{tag}_tf{u}")
    nc.vector.tensor_scalar(out=tf[:], in0=t[:], scalar1=0.0, scalar2=None, op0=Alu.add)
    fix = pool.tile([p, f], out_dtype, tag=f"{tag}_fix", name=f"{tag}_fix{u}")
    nc.vector.tensor_tensor(out=fix[:], in0=tf[:], in1=y[:], op=Alu.is_gt)
    b = pool.tile([p, f], out_dtype, tag=f"{tag}_b", name=f"{tag}_b{u}")
    nc.vector.tensor_tensor(out=b[:], in0=t[:], in1=fix[:], op=Alu.subtract)
    return b


@with_exitstack
def clahe_kernel(ctx: ExitStack, tc: tile.TileContext, out_ap, in_ap, dbg=None,
                 phase_max=4):
    nc = tc.nc
    x_hb = in_ap.rearrange("(n p) w -> n p w", p=P)     # [16, 128, 2048]
    out_hb = out_ap.rearrange("(n p) w -> n p w", p=P)

    misc = ctx.enter_context(tc.tile_pool(name="misc", bufs=1))
    hist64 = misc.tile([64, NB], dt.float32)  # per-tile histograms

    # ---------------- Phase 1: histograms ----------------
    HW2 = 1024  # col-half width
    with tc.tile_pool(name="io", bufs=2) as io, \
         tc.tile_pool(name="oh", bufs=1) as oh, \
         tc.tile_pool(name="psum", bufs=1, space="PSUM") as psum:
        for r in range(GH):  # tile-row
            ptiles = [psum.tile([16, 16], dt.float32, tag=f"ph{t_}", name=f"ph{r}_{t_}")
                      for t_ in range(8)]
            for hbi, hb in enumerate((2 * r, 2 * r + 1)):
                xt = io.tile([P, W], dt.float32, tag="x", name=f"x{hb}")
                nc.sync.dma_start(xt[:], x_hb[hb])
                y = io.tile([P, W], dt.float32, tag="y256", name=f"y{hb}")
                nc.vector.tensor_scalar(out=y[:], in0=xt[:], scalar1=256.0,
                                        scalar2=None, op0=Alu.mult)
                b16 = floor_pass(nc, io, y, "bh")  # [128, 2048] int16
                bhi = io.tile([P, W], dt.int16, tag="bhi", name=f"bhi{hb}")
                nc.vector.tensor_scalar(out=bhi[:], in0=b16[:], scalar1=4,
                                        scalar2=None, op0=Alu.logical_shift_right)
                blo = io.tile([P, W], dt.int16, tag="blo", name=f"blo{hb}")
                nc.vector.tensor_scalar(out=blo[:], in0=b16[:], scalar1=15,
                                        scalar2=None, op0=Alu.bitwise_and)
                for chh in range(2):  # col-half
                    A = oh.tile([P, HW2, 16], dt.float8e4, tag="A", name=f"A{hb}_{chh}")
                    B = oh.tile([P, HW2, 16], dt.float8e4, tag="B", name=f"B{hb}_{chh}")
                    hsl = bhi[:, chh * HW2:(chh + 1) * HW2]
                    lsl = blo[:, chh * HW2:(chh + 1) * HW2]
                    for h in range(16):
                        nc.vector.tensor_scalar(out=A[:, :, h], in0=hsl, scalar1=h,
                                                scalar2=None, op0=Alu.is_equal)
                    for l in range(16):
                        nc.vector.tensor_scalar(out=B[:, :, l], in0=lsl, scalar1=l,
                                                scalar2=None, op0=Alu.is_equal)
                    for step in range(128):
                        for tci in range(4):
                            c = tci * 256 + 2 * step
                            col = chh * HW2 + c
                            tcol = col >> 8
                            first = (hbi == 0) and step == 0
                            last = (hbi == 1) and step == 127
                            nc.tensor.matmul(
                                out=ptiles[tcol][:],
                                lhsT=A[:, c:c + 2, :], rhs=B[:, c:c + 2, :],
                                perf_mode=mybir.MatmulPerfMode.DoubleRow,
                                start=first, stop=last)
            # evacuate tile-row histograms
            for tcol in range(8):
                t = 8 * r + tcol
                stg = io.tile([16, 16], dt.float32, tag="hstg", name=f"hstg{r}_{tcol}")
                nc.vector.tensor_copy(stg[:], ptiles[tcol][:])
                nc.sync.dma_start(
                    hist64[t:t + 1, :].rearrange("a (b c) -> a b c", b=16), stg[:])

    if phase_max == 1:
        nc.sync.dma_start(dbg[:64, :NB], hist64[:])
        return

    # ---------------- Phase 2: per-tile LUT ----------------
    iota_i = misc.tile([64, NB], dt.int32)
    nc.gpsimd.iota(iota_i[:], pattern=[[1, NB]], base=0, channel_multiplier=0)
    iota_f = misc.tile([64, NB], dt.float32)
    nc.vector.tensor_scalar(out=iota_f[:], in0=iota_i[:], scalar1=0.0, scalar2=None,
                            op0=Alu.add)

    m = misc.tile([64, NB], dt.float32)
    total = misc.tile([64, 1], dt.float32)
    nc.vector.tensor_scalar(out=m[:], in0=hist64[:], scalar1=MAXVAL, scalar2=0.0,
                            op0=Alu.min, op1=Alu.add, accum_out=total[:])
    clipped = misc.tile([64, 1], dt.float32)
    nc.vector.tensor_scalar(out=clipped[:], in0=total[:], scalar1=-1.0,
                            scalar2=float(PIXELS), op0=Alu.mult, op1=Alu.add)
    # redist = floor(clipped/256); residual = clipped - 256*redist
    q = misc.tile([64, 1], dt.float32)
    nc.vector.tensor_scalar(out=q[:], in0=clipped[:], scalar1=1.0 / NB, scalar2=None,
                            op0=Alu.mult)
    redq = floor_pass(nc, misc, q, "redq", out_dtype=dt.int32)
    redist = misc.tile([64, 1], dt.float32)
    nc.vector.tensor_scalar(out=redist[:], in0=redq[:], scalar1=0.0, scalar2=None,
                            op0=Alu.add)
    residual = misc.tile([64, 1], dt.float32)
    nc.vector.scalar_tensor_tensor(out=residual[:], in0=redist[:], scalar=-float(NB),
                                   in1=clipped[:], op0=Alu.mult, op1=Alu.add)
    # m3 = m + redist + (iota < residual)
    t1 = misc.tile([64, NB], dt.float32)
    nc.vector.scalar_tensor_tensor(out=t1[:], in0=iota_f[:], scalar=residual[:],
                                   in1=m[:], op0=Alu.is_lt, op1=Alu.add)
    m3 = misc.tile([64, NB], dt.float32)
    nc.vector.tensor_scalar(out=m3[:], in0=t1[:], scalar1=redist[:], scalar2=None,
                            op0=Alu.add)
    zeros = misc.tile([64, NB], dt.float32)
    nc.vector.memset(zeros[:], 0.0)
    cdf = misc.tile([64, NB], dt.float32)
    nc.vector.tensor_tensor_scan(out=cdf[:], data0=m3[:], data1=zeros[:],
                                 initial=0.0, op0=Alu.add, op1=Alu.add)
    lutf = misc.tile([64, NB], dt.float32)
    nc.vector.tensor_scalar(out=lutf[:], in0=cdf[:], scalar1=255.0 / PIXELS,
                            scalar2=None, op0=Alu.mult)
    lut_i = floor_pass(nc, misc, lutf, "lut", out_dtype=dt.int32)  # [64, 256] int32

    if phase_max == 2:
        lut_f = misc.tile([64, NB], dt.float32)
        nc.vector.tensor_scalar(out=lut_f[:], in0=lut_i[:], scalar1=0.0, scalar2=None,
                                op0=Alu.add)
        nc.sync.dma_start(dbg[:64, :NB], lut_f[:])
        return

    # ---------------- Phase 3: packed region tables ----------------
    p3pool = ctx.enter_context(tc.tile_pool(name="p3", bufs=1))
    quad = p3pool.tile([81, 4, NB], dt.int32)  # [region, cfg, bin]
    for ry in range(9):
        ty0, ty1 = max(ry - 1, 0), min(ry, 7)
        for rx in range(9):
            tx0, tx1 = max(rx - 1, 0), min(rx, 7)
            reg = 9 * ry + rx
            for cfg, (ty, tx) in enumerate(
                    [(ty0, tx0), (ty0, tx1), (ty1, tx0), (ty1, tx1)]):
                nc.sync.dma_start(quad[reg:reg + 1, cfg, :],
                                  lut_i[8 * ty + tx:8 * ty + tx + 1, :])
    packed = p3pool.tile([81, NB], dt.int32)
    u1 = p3pool.tile([81, NB], dt.int32)
    nc.vector.tensor_scalar(out=u1[:], in0=quad[:81, 1, :], scalar1=8, scalar2=None,
                            op0=Alu.logical_shift_left)
    u2 = p3pool.tile([81, NB], dt.int32)
    nc.vector.tensor_scalar(out=u2[:], in0=quad[:81, 2, :], scalar1=16, scalar2=None,
                            op0=Alu.logical_shift_left)
    u3 = p3pool.tile([81, NB], dt.int32)
    nc.vector.tensor_scalar(out=u3[:], in0=quad[:81, 3, :], scalar1=24, scalar2=None,
                            op0=Alu.logical_shift_left)
    v1 = p3pool.tile([81, NB], dt.int32)
    nc.vector.tensor_tensor(out=v1[:], in0=quad[:81, 0, :], in1=u1[:], op=Alu.bitwise_or)
    v2 = p3pool.tile([81, NB], dt.int32)
    nc.vector.tensor_tensor(out=v2[:], in0=u2[:], in1=u3[:], op=Alu.bitwise_or)
    nc.vector.tensor_tensor(out=packed[:], in0=v1[:], in1=v2[:], op=Alu.bitwise_or)

    pdram = nc.dram_tensor("ptab", [81, NB], dt.int32).ap()
    nc.sync.dma_start(pdram[:, :], packed[:])

    # ---------------- static blend patterns ----------------
    # wx along columns (persistent [128, 2048] f32 x2)
    wx = misc.tile([P, W], dt.float32)
    wxm1 = misc.tile([P, W], dt.float32)
    with tc.tile_pool(name="scr", bufs=1) as scr:
        si = scr.tile([P, W], dt.int32, tag="si", name="si")
        nc.gpsimd.iota(si[:], pattern=[[1, W]], base=0, channel_multiplier=0)
        sf1 = scr.tile([P, W], dt.float32, tag="sf1", name="sf1")
        nc.vector.tensor_scalar(out=sf1[:], in0=si[:], scalar1=0.0, scalar2=None,
                                op0=Alu.add)
        sf2 = scr.tile([P, W], dt.float32, tag="sf2", name="sf2")
        nc.vector.tensor_scalar(out=sf2[:], in0=sf1[:], scalar1=1.0 / TW,
                                scalar2=0.5 / TW - 0.5, op0=Alu.mult, op1=Alu.add)
        si2 = scr.tile([P, W], dt.int32, tag="si2", name="si2")
        nc.vector.tensor_scalar(out=si2[:], in0=sf2[:], scalar1=0.0, scalar2=None,
                                op0=Alu.add)
        sf3 = scr.tile([P, W], dt.float32, tag="sf3", name="sf3")
        nc.vector.tensor_scalar(out=sf3[:], in0=si2[:], scalar1=0.0, scalar2=None,
                                op0=Alu.add)
        sf4 = scr.tile([P, W], dt.float32, tag="sf4", name="sf4")
        nc.vector.tensor_tensor(out=sf4[:], in0=sf3[:], in1=sf2[:], op=Alu.is_gt)
        sf5 = scr.tile([P, W], dt.float32, tag="sf5", name="sf5")
        nc.vector.tensor_tensor(out=sf5[:], in0=sf3[:], in1=sf4[:], op=Alu.subtract)
        nc.vector.tensor_tensor(out=wx[:], in0=sf2[:], in1=sf5[:], op=Alu.subtract)
        nc.vector.tensor_scalar(out=wxm1[:], in0=wx[:], scalar1=-1.0, scalar2=1.0,
                                op0=Alu.mult, op1=Alu.add)

    # wy per-partition per half-band: [128, 16] tiny
    wy_all = misc.tile([P, NHB], dt.float32)
    wym1_all = misc.tile([P, NHB], dt.float32)
    ri2 = misc.tile([P, NHB], dt.int32)
    nc.gpsimd.iota(ri2[:], pattern=[[128, NHB]], base=0, channel_multiplier=1)
    rf = misc.tile([P, NHB], dt.float32)
    nc.vector.tensor_scalar(out=rf[:], in0=ri2[:], scalar1=0.0, scalar2=None, op0=Alu.add)
    ty_ = misc.tile([P, NHB], dt.float32)
    nc.vector.tensor_scalar(out=ty_[:], in0=rf[:], scalar1=1.0 / TH,
                            scalar2=0.5 / TH - 0.5, op0=Alu.mult, op1=Alu.add)
    tyi = misc.tile([P, NHB], dt.int32)
    nc.vector.tensor_scalar(out=tyi[:], in0=ty_[:], scalar1=0.0, scalar2=None, op0=Alu.add)
    tyif = misc.tile([P, NHB], dt.float32)
    nc.vector.tensor_scalar(out=tyif[:], in0=tyi[:], scalar1=0.0, scalar2=None, op0=Alu.add)
    fixy = misc.tile([P, NHB], dt.float32)
    nc.vector.tensor_tensor(out=fixy[:], in0=tyif[:], in1=ty_[:], op=Alu.is_gt)
    y0f = misc.tile([P, NHB], dt.float32)
    nc.vector.tensor_tensor(out=y0f[:], in0=tyif[:], in1=fixy[:], op=Alu.subtract)
    nc.vector.tensor_tensor(out=wy_all[:], in0=ty_[:], in1=y0f[:], op=Alu.subtract)
    nc.vector.tensor_scalar(out=wym1_all[:], in0=wy_all[:], scalar1=-1.0, scalar2=1.0,
                            op0=Alu.mult, op1=Alu.add)

    # ---------------- Phase 4: apply ----------------
    with tc.tile_pool(name="tabs", bufs=2) as tabs, \
         tc.tile_pool(name="app", bufs=1) as app, \
         tc.tile_pool(name="bl", bufs=1) as bl:
        cur_tab = None
        cur_ry = -1
        for hb in range(NHB):
            ry = RY_OF_HB[hb]
            if ry != cur_ry:
                cur_tab = tabs.tile([P, 16, NB], dt.int32, tag="ptab",
                                    name=f"ptab{ry}")
                for hs in range(16):
                    reg = 9 * ry + RX_OF_HS[hs]
                    nc.sync.dma_start(
                        cur_tab[:, hs, :],
                        pdram[reg:reg + 1, :].to_broadcast([P, NB]))
                cur_ry = ry
            xt = app.tile([P, W], dt.float32, tag="ax", name=f"ax{hb}")
            nc.sync.dma_start(xt[:], x_hb[hb])
            yp = app.tile([P, W], dt.float32, tag="ay", name=f"ay{hb}")
            nc.vector.tensor_scalar(out=yp[:], in0=xt[:], scalar1=255.0, scalar2=None,
                                    op0=Alu.mult)
            bp = floor_pass(nc, app, yp, "bp")  # [128, 2048] int16

            acc = app.tile([P, W], dt.int32, tag="acc", name=f"acc{hb}")
            for k in range(NB):
                mask = app.tile([P, W], dt.int16, tag="mask", name=f"mk{hb}_{k}",
                                bufs=2)
                nc.vector.tensor_scalar(out=mask[:], in0=bp[:], scalar1=k,
                                        scalar2=None, op0=Alu.is_equal)
                data_ap = cur_tab[:, :, k:k + 1].to_broadcast([P, 16, P])
                nc.vector.copy_predicated(
                    out=acc[:].rearrange("p (a c) -> p a c", a=16),
                    mask=mask[:].rearrange("p (a c) -> p a c", a=16),
                    data=data_ap)
            # unpack + blend in col-halves
            for chh in range(2):
                sl = slice(chh * HW2, (chh + 1) * HW2)
                asl = acc[:, sl]
                vs = []
                for vi, (sh, use_and) in enumerate([(0, True), (8, True), (16, True), (24, False)]):
                    wi = bl.tile([P, HW2], dt.int32, tag=f"w{vi}", name=f"w{vi}_{hb}_{chh}")
                    if sh == 0:
                        nc.vector.tensor_scalar(out=wi[:], in0=asl, scalar1=255,
                                                scalar2=None, op0=Alu.bitwise_and)
                    elif use_and:
                        nc.vector.tensor_scalar(out=wi[:], in0=asl, scalar1=sh, scalar2=255,
                                                op0=Alu.logical_shift_right,
                                                op1=Alu.bitwise_and)
                    else:
                        nc.vector.tensor_scalar(out=wi[:], in0=asl, scalar1=sh,
                                                scalar2=None, op0=Alu.logical_shift_right)
                    vf = bl.tile([P, HW2], dt.float32, tag=f"v{vi}", name=f"v{vi}_{hb}_{chh}")
                    nc.vector.tensor_scalar(out=vf[:], in0=wi[:], scalar1=0.0,
                                            scalar2=None, op0=Alu.add)
                    vs.append(vf)
                v00, v01, v10, v11 = vs
                p1 = bl.tile([P, HW2], dt.float32, tag="p1", name=f"p1_{hb}_{chh}")
                nc.vector.tensor_tensor(out=p1[:], in0=v00[:], in1=wxm1[:, sl],
                                        op=Alu.mult)
                p2 = bl.tile([P, HW2], dt.float32, tag="p2", name=f"p2_{hb}_{chh}")
                nc.vector.tensor_tensor(out=p2[:], in0=v01[:], in1=wx[:, sl],
                                        op=Alu.mult)
                top = bl.tile([P, HW2], dt.float32, tag="top", name=f"top_{hb}_{chh}")
                nc.vector.tensor_tensor(out=top[:], in0=p1[:], in1=p2[:], op=Alu.add)
                p3 = bl.tile([P, HW2], dt.float32, tag="p3", name=f"p3_{hb}_{chh}")
                nc.vector.tensor_tensor(out=p3[:], in0=v10[:], in1=wxm1[:, sl],
                                        op=Alu.mult)
                p4 = bl.tile([P, HW2], dt.float32, tag="p4", name=f"p4_{hb}_{chh}")
                nc.vector.tensor_tensor(out=p4[:], in0=v11[:], in1=wx[:, sl],
                                        op=Alu.mult)
                bot = bl.tile([P, HW2], dt.float32, tag="bot", name=f"bot_{hb}_{chh}")
                nc.vector.tensor_tensor(out=bot[:], in0=p3[:], in1=p4[:], op=Alu.add)
                st = bl.tile([P, HW2], dt.float32, tag="st", name=f"st_{hb}_{chh}")
                nc.vector.tensor_scalar(out=st[:], in0=top[:],
                                        scalar1=wym1_all[:, hb:hb + 1], scalar2=None,
                                        op0=Alu.mult)
                sb = bl.tile([P, HW2], dt.float32, tag="sb", name=f"sb_{hb}_{chh}")
                nc.vector.tensor_scalar(out=sb[:], in0=bot[:],
                                        scalar1=wy_all[:, hb:hb + 1], scalar2=None,
                                        op0=Alu.mult)
                res = bl.tile([P, HW2], dt.float32, tag="res", name=f"res_{hb}_{chh}")
                nc.vector.tensor_tensor(out=res[:], in0=st[:], in1=sb[:], op=Alu.add)
                reso = bl.tile([P, HW2], dt.float32, tag="reso", name=f"reso_{hb}_{chh}")
                nc.vector.tensor_scalar(out=reso[:], in0=res[:], scalar1=1.0 / 255.0,
                                        scalar2=None, op0=Alu.mult)
                nc.sync.dma_start(out_hb[hb][:, sl], reso[:])


def build(phase_max=4):
    nc = bacc.Bacc("TRN2", target_bir_lowering=False, debug=False, num_devices=8)
    in_t = nc.dram_tensor("x", [H, W], dt.float32, kind="ExternalInput").ap()
    out_t = nc.dram_tensor("out", [H, W], dt.float32, kind="ExternalOutput").ap()
    dbg = None
    if phase_max < 4:
        dbg = nc.dram_tensor("dbg", [128, NB], dt.float32, kind="ExternalOutput").ap()
    with tile.TileContext(nc) as tc:
        clahe_kernel(tc, out_t, in_t, dbg=dbg, phase_max=phase_max)
    nc.compile()
    return nc


# ======================================================================
# Harness-facing entry point
# ======================================================================
import numpy as np

_NC_CACHE = {}


def _get_nc():
    if "nc" not in _NC_CACHE:
        _NC_CACHE["nc"] = build(phase_max=4)
    return _NC_CACHE["nc"]


def kernel(x: np.ndarray) -> np.ndarray:
    """CLAHE on (8, 1, 2048, 2048) fp32; batch sharded across 8 NeuronCores."""
    from concourse.bass_utils import run_bass_kernel_spmd

    x = np.asarray(x, dtype=np.float32)
    B, C, Hh, Ww = x.shape
    assert (B, C, Hh, Ww) == (8, 1, 2048, 2048), x.shape
    nc = _get_nc()
    in_maps = [{"x": np.ascontiguousarray(x[i, 0])} for i in range(B)]
    res = run_bass_kernel_spmd(nc, in_maps, list(range(8)))
    out = np.stack([np.asarray(res.results[i]["out"]) for i in range(B)])[:, None]
    return out.astype(np.float32)

